# revision 5
# baseline (speedup 1.0000x reference)
"""Trainium2 Bass kernel for NTreeLSTM (complete binary tree, heap layout).

Strategy (self-contained; shapes hardcoded):
- Global tree: LEVELS=18, N=262143, H=256. Core m (of 8) owns the subtree
  rooted at global level-3 node 7+m. All child gathers are core-local.
- Only leaf rows of x matter (internal init is overwritten before use), so
  each core receives just its 16384 contiguous leaf rows of x.
- On-device state is feature-major (h^T/c^T: [feat partition, node free]) so
  biases ride the ACT engine's per-partition bias and the per-level child
  gather is a stride-2 access pattern fed straight to the PE.
- Per core: 8 chunks × (leaf init + levels 13..6), tail levels 5..0, output
  h for all 32767 local nodes (+ local root c) via PE transpose.
- Host: gathers leaf shards, scatters per-level outputs back to the global
  heap, and computes the top 7 nodes (global levels 2..0) in numpy.
"""
import sys

if "/opt/trn_rl_repo" not in sys.path:
    sys.path.insert(0, "/opt/trn_rl_repo")

import numpy as np

import concourse.bass as bass
import concourse.mybir as mybir
import concourse.tile as tile
from concourse import bacc
from concourse.masks import make_identity

FP = mybir.dt.float32
AF = mybir.ActivationFunctionType

H = 256
LEVELS = 18
N = 2**LEVELS - 1
NCORES = 8

G = 512          # node-group (matmul moving free dim)
NCHUNK = 8       # chunks per core
CB = 6           # lowest level processed per-chunk; tail handles CB-1..0


def _stride2(ap, parity):
    """Select every other free-dim column of a 2D [P, 2n] AP."""
    assert len(ap.ap) == 2, ap.ap
    (pstep, pcount), (fstep, fcount) = ap.ap
    assert fcount % 2 == 0
    return bass.AP(
        tensor=ap.tensor,
        offset=ap.offset + parity * fstep,
        ap=[[pstep, pcount], [2 * fstep, fcount // 2]],
    )


class _Emitter:
    def __init__(self, nc, pools, consts, out_d):
        self.nc = nc
        self.pools = pools
        self.c = consts
        self.out_d = out_d
        self._copy_flip = 0

    def _psum_copy(self, dst, src):
        # split psum->sbuf copies between DVE and ACT
        if self._copy_flip == 0:
            self.nc.vector.tensor_copy(dst, src)
        else:
            self.nc.scalar.copy(dst, src)
        self._copy_flip ^= 1

    def emit_rows_out(self, src, col0, w, row0):
        """Transpose src[:, t, col0:col0+w] (feature-major) to node-major DRAM
        rows out_d[row0:row0+w]."""
        nc, p = self.nc, self.pools
        for j0 in range(0, w, 128):
            gw = min(128, w - j0)
            ps = p["psum"].tile([128, 256], FP, tag="psout", bufs=2)
            nc.tensor.transpose(
                ps[:gw, 0:128], src[:, 0, col0 + j0:col0 + j0 + gw], self.c["ident"])
            nc.tensor.transpose(
                ps[:gw, 128:256], src[:, 1, col0 + j0:col0 + j0 + gw], self.c["ident"])
            osb = p["out"].tile([128, 256], FP, tag="osb", bufs=4)
            self._psum_copy(osb[:gw, :], ps[:gw, :])
            nc.sync.dma_start(
                self.out_d[row0 + j0:row0 + j0 + gw, :], osb[:gw, :])

    def emit_leaf_init(self, x_d, xrow0, nleaf, h14, c14, dst0, out_row0):
        """init = tanh(x @ W + b) for nleaf leaves; h/c into h14/c14 cols
        [dst0, dst0+nleaf); h also to DRAM rows [out_row0, ...)."""
        nc, p, c = self.nc, self.pools, self.c
        for g0 in range(0, nleaf, G):
            gw = min(G, nleaf - g0)
            ngr = (gw + 127) // 128
            xg = p["x"].tile([128, ngr, 256], FP, tag="xg", bufs=2)
            nc.sync.dma_start(
                xg[:, :, :],
                x_d[xrow0 + g0:xrow0 + g0 + gw, :].rearrange(
                    "(j p) f -> p j f", p=128))
            xt = p["x"].tile([128, ngr, 2, 128], FP, tag="xt", bufs=2)
            for j in range(ngr):
                ps = p["psum"].tile([128, 256], FP, tag="pstr", bufs=2)
                nc.tensor.transpose(ps[:, 0:128], xg[:, j, 0:128], c["ident"])
                nc.tensor.transpose(ps[:, 128:256], xg[:, j, 128:256], c["ident"])
                self._psum_copy(xt[:, j, :, :], ps[:, :])
            for m in range(4):
                ps = p["psum"].tile([128, G], FP, tag="psmm", bufs=4)
                for kt in range(2):
                    nc.tensor.matmul(
                        ps[:, :gw],
                        c["w"][:, kt, 128 * m:128 * m + 128],
                        xt[:, :, kt, :],
                        start=(kt == 0), stop=(kt == 1))
                dst = h14 if m < 2 else c14
                t = m % 2
                nc.scalar.activation(
                    dst[:, t, dst0 + g0:dst0 + g0 + gw], ps[:, :gw],
                    AF.Tanh, bias=c["wb"][:, m:m + 1])
            self.emit_rows_out(h14, dst0 + g0, gw, out_row0 + g0)

    def emit_level(self, h_ch, c_ch, ch0, h_par, c_par, dst0, w, out_row0):
        """TreeLSTM cell for w parents whose children are cols
        [ch0, ch0+2w) of h_ch/c_ch; writes parent cols [dst0, dst0+w)."""
        nc, p, c = self.nc, self.pools, self.c
        for g0 in range(0, w, G):
            gw = min(G, w - g0)
            co = ch0 + 2 * (g0)

            def rhs(kt):
                par, t = (0, kt) if kt < 2 else (1, kt - 2)
                return _stride2(h_ch[:, t, co:co + 2 * gw], par)

            iou = p["gates"].tile([128, 6, G], FP, tag="iou", bufs=2)
            for m in range(6):
                ps = p["psum"].tile([128, G], FP, tag="psmm", bufs=4)
                for kt in range(4):
                    nc.tensor.matmul(
                        ps[:, :gw],
                        c["uiou"][:, kt, 128 * m:128 * m + 128],
                        rhs(kt), start=(kt == 0), stop=(kt == 3))
                nc.scalar.activation(
                    iou[:, m, :gw], ps[:, :gw],
                    AF.Tanh if m >= 4 else AF.Sigmoid,
                    bias=c["uioub"][:, m:m + 1])
            f = p["gates"].tile([128, 4, G], FP, tag="f", bufs=2)
            for m in range(4):
                ps = p["psum"].tile([128, G], FP, tag="psmm", bufs=4)
                for kt in range(4):
                    nc.tensor.matmul(
                        ps[:, :gw],
                        c["uf"][:, kt, 128 * m:128 * m + 128],
                        rhs(kt), start=(kt == 0), stop=(kt == 3))
                nc.scalar.activation(
                    f[:, m, :gw], ps[:, :gw], AF.Sigmoid,
                    bias=c["ufb"][:, m:m + 1])
            for t in range(2):
                cdst = c_par[:, t, dst0 + g0:dst0 + g0 + gw]
                c_ev = _stride2(c_ch[:, t, co:co + 2 * gw], 0)
                c_od = _stride2(c_ch[:, t, co:co + 2 * gw], 1)
                t1 = p["gates"].tile([128, G], FP, tag="tmp1", bufs=2)
                t2 = p["gates"].tile([128, G], FP, tag="tmp2", bufs=2)
                nc.vector.tensor_mul(cdst, f[:, t, :gw], c_ev)
                nc.vector.tensor_mul(t1[:, :gw], f[:, 2 + t, :gw], c_od)
                nc.vector.tensor_mul(t2[:, :gw], iou[:, t, :gw], iou[:, 4 + t, :gw])
                nc.vector.tensor_add(cdst, cdst, t1[:, :gw])
                nc.vector.tensor_add(cdst, cdst, t2[:, :gw])
                th = p["gates"].tile([128, G], FP, tag="tanhc", bufs=2)
                nc.scalar.activation(th[:, :gw], cdst, AF.Tanh)
                nc.vector.tensor_mul(
                    h_par[:, t, dst0 + g0:dst0 + g0 + gw],
                    iou[:, 2 + t, :gw], th[:, :gw])
            self.emit_rows_out(h_par, dst0 + g0, gw, out_row0 + g0)


def build_nc(llev=15):
    """Build the single-core program. Local tree: levels 0..llev-1, leaves at
    local level llev-1; identical on all 8 cores (SPMD, no collectives)."""
    leaf_lvl = llev - 1
    nleaves = 2**leaf_lvl
    lnodes = 2**llev - 1
    chunk_leaves = nleaves // NCHUNK

    nc = bacc.Bacc(None, target_bir_lowering=False)
    x_d = nc.dram_tensor("x", [nleaves, 256], FP, kind="ExternalInput")
    w_d = nc.dram_tensor("w_w", [256, 512], FP, kind="ExternalInput")
    wb_d = nc.dram_tensor("w_b", [512], FP, kind="ExternalInput")
    uiou_d = nc.dram_tensor("uiou_w", [512, 768], FP, kind="ExternalInput")
    uioub_d = nc.dram_tensor("uiou_b", [768], FP, kind="ExternalInput")
    uf_d = nc.dram_tensor("uf_w", [512, 512], FP, kind="ExternalInput")
    ufb_d = nc.dram_tensor("ufb", [512], FP, kind="ExternalInput")
    out_d = nc.dram_tensor("out", [lnodes + 1, 256], FP, kind="ExternalOutput")

    with tile.TileContext(nc) as tc:
        with (
            tc.tile_pool(name="const", bufs=1) as const,
            tc.tile_pool(name="state", bufs=1) as state,
            tc.tile_pool(name="x", bufs=1) as xpool,
            tc.tile_pool(name="gates", bufs=1) as gates,
            tc.tile_pool(name="out", bufs=1) as outp,
            tc.tile_pool(name="psum", bufs=1, space="PSUM") as psum,
        ):
            consts = {}
            consts["ident"] = const.tile([128, 128], FP, tag="ident", name="ident")
            make_identity(nc, consts["ident"])
            consts["w"] = const.tile([128, 2, 512], FP, tag="w", name="w")
            nc.sync.dma_start(
                consts["w"][:], w_d[:].rearrange("(kt p) m -> p kt m", p=128))
            consts["uiou"] = const.tile([128, 4, 768], FP, tag="uiou", name="uiou")
            nc.sync.dma_start(
                consts["uiou"][:], uiou_d[:].rearrange("(kt p) m -> p kt m", p=128))
            consts["uf"] = const.tile([128, 4, 512], FP, tag="uf", name="uf")
            nc.sync.dma_start(
                consts["uf"][:], uf_d[:].rearrange("(kt p) m -> p kt m", p=128))
            consts["wb"] = const.tile([128, 4], FP, tag="wb", name="wb")
            nc.sync.dma_start(
                consts["wb"][:], wb_d[:].rearrange("(m p) -> p m", p=128))
            consts["uioub"] = const.tile([128, 6], FP, tag="uioub", name="uioub")
            nc.sync.dma_start(
                consts["uioub"][:], uioub_d[:].rearrange("(m p) -> p m", p=128))
            consts["ufb"] = const.tile([128, 4], FP, tag="ufb", name="ufb")
            nc.sync.dma_start(
                consts["ufb"][:], ufb_d[:].rearrange("(m p) -> p m", p=128))

            pools = {"psum": psum, "x": xpool, "gates": gates, "out": outp}
            em = _Emitter(nc, pools, consts, out_d)

            # resident state for level CB (written by chunks, read by tail)
            wcb = 2**CB
            hcb = const.tile([128, 2, wcb], FP, tag="hcb", name="hcb")
            ccb = const.tile([128, 2, wcb], FP, tag="ccb", name="ccb")

            for ch in range(NCHUNK):
                # per-chunk state tiles, levels leaf_lvl..CB+1
                hs, cs = {}, {}
                for k in range(CB + 1, leaf_lvl + 1):
                    wk = chunk_leaves >> (leaf_lvl - k)
                    nb = 2 if k == leaf_lvl else 1
                    hs[k] = state.tile([128, 2, wk], FP, tag=f"h{k}", bufs=nb, name=f"h{k}_{ch}")
                    cs[k] = state.tile([128, 2, wk], FP, tag=f"c{k}", bufs=nb, name=f"c{k}_{ch}")
                hs[CB], cs[CB] = hcb, ccb

                em.emit_leaf_init(
                    x_d, ch * chunk_leaves, chunk_leaves,
                    hs[leaf_lvl], cs[leaf_lvl], 0,
                    (2**leaf_lvl - 1) + ch * chunk_leaves)
                for k in range(leaf_lvl - 1, CB - 1, -1):
                    wk = chunk_leaves >> (leaf_lvl - k)
                    dst0 = ch * wk if k == CB else 0
                    em.emit_level(
                        hs[k + 1], cs[k + 1], 0, hs[k], cs[k], dst0, wk,
                        (2**k - 1) + ch * wk)

            # tail: levels CB-1 .. 0
            hprev, cprev = hcb, ccb
            for k in range(CB - 1, -1, -1):
                wk = 2**k
                hk = const.tile([128, 2, wk], FP, tag=f"ht{k}", name=f"ht{k}")
                ck = const.tile([128, 2, wk], FP, tag=f"ct{k}", name=f"ct{k}")
                em.emit_level(hprev, cprev, 0, hk, ck, 0, wk, 2**k - 1)
                hprev, cprev = hk, ck
            # local root c -> last output row
            em.emit_rows_out(cprev, 0, 1, lnodes)

    nc.compile()
    return nc


# ---------------- host side ----------------

_CACHE = {}
LAST_RESULTS = None


def _sigmoid(v):
    return 1.0 / (1.0 + np.exp(-v))


def _cell_np(ch_h, ch_c, Uiou_w, Uiou_b, Uf_w, Uf_b):
    f = _sigmoid(ch_h @ Uf_w + Uf_b).reshape(-1, 2, H)
    iou = ch_h @ Uiou_w + Uiou_b
    i, o, u = np.split(iou, 3, axis=1)
    i, o, u = _sigmoid(i), _sigmoid(o), np.tanh(u)
    c = i * u + (f * ch_c).sum(axis=1)
    h = o * np.tanh(c)
    return h.astype(np.float32), c.astype(np.float32)


def kernel(x, W_w, W_b, Uiou_w, Uiou_b, Uf_w, Uf_b):
    global LAST_RESULTS
    from concourse.bass_utils import run_bass_kernel_spmd

    llev = LEVELS - 3
    if "nc" not in _CACHE:
        _CACHE["nc"] = build_nc(llev)
    nc = _CACHE["nc"]

    f32 = lambda a: np.ascontiguousarray(np.asarray(a, dtype=np.float32))
    x = f32(x)
    weights = {
        "w_w": f32(W_w), "w_b": f32(W_b), "uiou_w": f32(Uiou_w),
        "uiou_b": f32(Uiou_b), "uf_w": f32(Uf_w), "ufb": f32(Uf_b),
    }
    nleaves = 2**(llev - 1)
    lnodes = 2**llev - 1
    leaf0 = 2**(LEVELS - 1) - 1
    in_maps = []
    for m in range(NCORES):
        im = dict(weights)
        im["x"] = x[leaf0 + nleaves * m: leaf0 + nleaves * (m + 1)]
        in_maps.append(im)

    res = run_bass_kernel_spmd(nc, in_maps, core_ids=list(range(NCORES)))
    LAST_RESULTS = res

    out = np.zeros((N, H), np.float32)
    roots_h = np.zeros((NCORES, H), np.float32)
    roots_c = np.zeros((NCORES, H), np.float32)
    for m in range(NCORES):
        hm = res.results[m]["out"]
        for k in range(llev):
            goff = 2**(k + 3) - 1 + m * 2**k
            loff = 2**k - 1
            out[goff:goff + 2**k] = hm[loff:loff + 2**k]
        roots_h[m] = hm[0]
        roots_c[m] = hm[lnodes]
    # top 7 nodes (global levels 2..0) on host
    h_top = np.zeros((7, H), np.float32)
    c_top = np.zeros((7, H), np.float32)
    ch_h = np.concatenate([roots_h[0::2], roots_h[1::2]], axis=1)
    ch_c = np.stack([roots_c[0::2], roots_c[1::2]], axis=1)
    h_top[3:7], c_top[3:7] = _cell_np(ch_h, ch_c, weights["uiou_w"],
                                      weights["uiou_b"], weights["uf_w"],
                                      weights["ufb"])
    for lvl in (1, 0):
        off, cnt, coff = 2**lvl - 1, 2**lvl, 2**(lvl + 1) - 1
        ch_h = h_top[coff:coff + 2 * cnt].reshape(cnt, 2 * H)
        ch_c = c_top[coff:coff + 2 * cnt].reshape(cnt, 2, H)
        h_top[off:off + cnt], c_top[off:off + cnt] = _cell_np(
            ch_h, ch_c, weights["uiou_w"], weights["uiou_b"],
            weights["uf_w"], weights["ufb"])
    out[0:7] = h_top
    return out


# revision 11
# speedup vs baseline: 1.6837x; 1.6837x over previous
"""Trainium2 Bass kernel for NTreeLSTM (complete binary tree, heap layout).

Strategy (self-contained; shapes hardcoded):
- Global tree: LEVELS=18, N=262143, H=256. Core m (of 8) owns the subtree
  rooted at global level-3 node 7+m. All child gathers are core-local.
- Only leaf rows of x matter (internal init is overwritten before use), so
  each core receives just its 16384 contiguous leaf rows of x.
- On-device state is feature-major (h^T/c^T: [feat partition, node free]) so
  biases ride the ACT engine's per-partition bias and the per-level child
  gather is a stride-2 access pattern fed straight to the PE.
- Per core: 8 chunks × (leaf init + levels 13..6), tail levels 5..0, output
  h for all 32767 local nodes (+ local root c) via PE transpose.
- Host: gathers leaf shards, scatters per-level outputs back to the global
  heap, and computes the top 7 nodes (global levels 2..0) in numpy.
"""
import sys

if "/opt/trn_rl_repo" not in sys.path:
    sys.path.insert(0, "/opt/trn_rl_repo")

import numpy as np

import concourse.bass as bass
import concourse.mybir as mybir
import concourse.tile as tile
from concourse import bacc
from concourse.masks import make_identity

FP = mybir.dt.float32
FR = mybir.dt.float32r
AF = mybir.ActivationFunctionType

H = 256
LEVELS = 18
N = 2**LEVELS - 1
NCORES = 8

G = 512          # node-group (matmul moving free dim)
NCHUNK = 8       # chunks per core
CB = 6           # lowest level processed per-chunk; tail handles CB-1..0


def _stride2(ap, parity):
    """Select every other free-dim column of a 2D [P, 2n] AP."""
    assert len(ap.ap) == 2, ap.ap
    (pstep, pcount), (fstep, fcount) = ap.ap
    assert fcount % 2 == 0
    return bass.AP(
        tensor=ap.tensor,
        offset=ap.offset + parity * fstep,
        ap=[[pstep, pcount], [2 * fstep, fcount // 2]],
    )


class _Emitter:
    def __init__(self, nc, pools, consts, out_d):
        self.nc = nc
        self.pools = pools
        self.c = consts
        self.out_d = out_d
        self._copy_flip = 0

    def _psum_copy(self, dst, src):
        # split psum->sbuf copies between DVE and ACT
        if self._copy_flip == 0:
            self.nc.vector.tensor_copy(dst, src)
        else:
            self.nc.scalar.copy(dst, src)
        self._copy_flip ^= 1

    def emit_rows_out(self, src, col0, w, row0):
        """Transpose src[:, t, col0:col0+w] (feature-major) to node-major DRAM
        rows out_d[row0:row0+w]."""
        nc, p = self.nc, self.pools
        for j0 in range(0, w, 128):
            gw = min(128, w - j0)
            ps = p["psum"].tile([128, 256], src.dtype, tag="psout", bufs=2)
            ident = self.c["identr" if src.dtype == FR else "ident"]
            nc.tensor.transpose(
                ps[:gw, 0:128], src[:, 0, col0 + j0:col0 + j0 + gw], ident)
            nc.tensor.transpose(
                ps[:gw, 128:256], src[:, 1, col0 + j0:col0 + j0 + gw], ident)
            osb = p["out"].tile([128, 256], FP, tag="osb", bufs=4)
            self._psum_copy(osb[:gw, :], ps[:gw, :])
            nc.sync.dma_start(
                self.out_d[row0 + j0:row0 + j0 + gw, :], osb[:gw, :])

    def emit_leaf_init(self, x_d, xrow0, nleaf, h14, c14, dst0, out_row0):
        """init = tanh(x @ W + b) for nleaf leaves; h/c into h14/c14 cols
        [dst0, dst0+nleaf); h also to DRAM rows [out_row0, ...)."""
        nc, p, c = self.nc, self.pools, self.c
        for g0 in range(0, nleaf, G):
            gw = min(G, nleaf - g0)
            ngr = (gw + 127) // 128
            xg = p["x"].tile([128, ngr, 256], FP, tag="xg", bufs=2)
            nc.sync.dma_start(
                xg[:, :, :],
                x_d[xrow0 + g0:xrow0 + g0 + gw, :].rearrange(
                    "(j p) f -> p j f", p=128))
            xt = p["x"].tile([128, ngr, 2, 128], FR, tag="xt", bufs=2)
            for j in range(ngr):
                ps = p["psum"].tile([128, 256], FP, tag="pstr", bufs=2)
                nc.tensor.transpose(ps[:, 0:128], xg[:, j, 0:128], c["ident"])
                nc.tensor.transpose(ps[:, 128:256], xg[:, j, 128:256], c["ident"])
                self._psum_copy(xt[:, j, :, :], ps[:, :])
            for m in range(4):
                ps = p["psum"].tile([128, G], FP, tag="psmm", bufs=4)
                for kt in range(2):
                    nc.tensor.matmul(
                        ps[:, :gw],
                        c["w"][:, kt, 128 * m:128 * m + 128],
                        xt[:, :, kt, :],
                        start=(kt == 0), stop=(kt == 1))
                dst = h14 if m < 2 else c14
                t = m % 2
                nc.scalar.activation(
                    dst[:, t, dst0 + g0:dst0 + g0 + gw], ps[:, :gw],
                    AF.Tanh, bias=c["wb"][:, m:m + 1])
            self.emit_rows_out(h14, dst0 + g0, gw, out_row0 + g0)

    def emit_level(self, h_ch, c_ch, ch0, h_par, c_par, dst0, w, out_row0):
        """TreeLSTM cell for w parents whose children are cols
        [ch0, ch0+2w) of h_ch/c_ch; writes parent cols [dst0, dst0+w)."""
        nc, p, c = self.nc, self.pools, self.c
        for g0 in range(0, w, G):
            gwr = min(G, w - g0)  # true width (DMA out)
            gw = gwr + gwr % 2    # fp32r ISA needs even free counts; padded
            co = ch0 + 2 * (g0)

            def rhs(kt):
                par, t = (0, kt) if kt < 2 else (1, kt - 2)
                return _stride2(h_ch[:, t, co:co + 2 * gw], par)

            iou = p["gates"].tile([128, 6, G], FP, tag="iou", bufs=2)
            for m in range(6):
                ps = p["psum"].tile([128, G], FP, tag="psmm", bufs=4)
                for kt in range(4):
                    nc.tensor.matmul(
                        ps[:, :gw],
                        c["uiou"][:, kt, 128 * m:128 * m + 128],
                        rhs(kt), start=(kt == 0), stop=(kt == 3))
                nc.scalar.activation(
                    iou[:, m, :gw], ps[:, :gw],
                    AF.Tanh if m >= 4 else AF.Sigmoid,
                    bias=c["uioub"][:, m:m + 1])
            f = p["gates"].tile([128, 4, G], FP, tag="f", bufs=2)
            for m in range(4):
                ps = p["psum"].tile([128, G], FP, tag="psmm", bufs=4)
                for kt in range(4):
                    nc.tensor.matmul(
                        ps[:, :gw],
                        c["uf"][:, kt, 128 * m:128 * m + 128],
                        rhs(kt), start=(kt == 0), stop=(kt == 3))
                nc.scalar.activation(
                    f[:, m, :gw], ps[:, :gw], AF.Sigmoid,
                    bias=c["ufb"][:, m:m + 1])
            for t in range(2):
                cdst = c_par[:, t, dst0 + g0:dst0 + g0 + gw]
                c_ev = _stride2(c_ch[:, t, co:co + 2 * gw], 0)
                c_od = _stride2(c_ch[:, t, co:co + 2 * gw], 1)
                t1 = p["gates"].tile([128, G], FP, tag="tmp1", bufs=2)
                t2 = p["gates"].tile([128, G], FP, tag="tmp2", bufs=2)
                nc.vector.tensor_mul(cdst, f[:, t, :gw], c_ev)
                nc.vector.tensor_mul(t1[:, :gw], f[:, 2 + t, :gw], c_od)
                nc.vector.tensor_mul(t2[:, :gw], iou[:, t, :gw], iou[:, 4 + t, :gw])
                nc.vector.tensor_add(cdst, cdst, t1[:, :gw])
                nc.vector.tensor_add(cdst, cdst, t2[:, :gw])
                th = p["gates"].tile([128, G], FP, tag="tanhc", bufs=2)
                nc.scalar.activation(th[:, :gw], cdst, AF.Tanh)
                nc.vector.tensor_mul(
                    h_par[:, t, dst0 + g0:dst0 + g0 + gw],
                    iou[:, 2 + t, :gw], th[:, :gw])
            self.emit_rows_out(h_par, dst0 + g0, gwr, out_row0 + g0)


def build_nc(llev=15):
    """Build the single-core program. Local tree: levels 0..llev-1, leaves at
    local level llev-1; identical on all 8 cores (SPMD, no collectives)."""
    leaf_lvl = llev - 1
    nleaves = 2**leaf_lvl
    lnodes = 2**llev - 1
    chunk_leaves = nleaves // NCHUNK

    nc = bacc.Bacc(None, target_bir_lowering=False)
    x_d = nc.dram_tensor("x", [nleaves, 256], FP, kind="ExternalInput")
    w_d = nc.dram_tensor("w_w", [256, 512], FP, kind="ExternalInput")
    wb_d = nc.dram_tensor("w_b", [512], FP, kind="ExternalInput")
    uiou_d = nc.dram_tensor("uiou_w", [512, 768], FP, kind="ExternalInput")
    uioub_d = nc.dram_tensor("uiou_b", [768], FP, kind="ExternalInput")
    uf_d = nc.dram_tensor("uf_w", [512, 512], FP, kind="ExternalInput")
    ufb_d = nc.dram_tensor("ufb", [512], FP, kind="ExternalInput")
    out_d = nc.dram_tensor("out", [lnodes + 1, 256], FP, kind="ExternalOutput")

    with tile.TileContext(nc) as tc:
        with (
            tc.tile_pool(name="const", bufs=1) as const,
            tc.tile_pool(name="state", bufs=1) as state,
            tc.tile_pool(name="x", bufs=1) as xpool,
            tc.tile_pool(name="gates", bufs=1) as gates,
            tc.tile_pool(name="out", bufs=1) as outp,
            tc.tile_pool(name="psum", bufs=1, space="PSUM") as psum,
        ):
            consts = {}
            consts["ident"] = const.tile([128, 128], FP, tag="ident", name="ident")
            make_identity(nc, consts["ident"])
            consts["identr"] = const.tile([128, 128], FR, tag="identr", name="identr")
            nc.vector.tensor_copy(consts["identr"][:], consts["ident"][:])
            consts["w"] = const.tile([128, 2, 512], FR, tag="w", name="w")
            nc.gpsimd.dma_start(
                consts["w"][:], w_d[:].rearrange("(kt p) m -> p kt m", p=128))
            consts["uiou"] = const.tile([128, 4, 768], FR, tag="uiou", name="uiou")
            nc.gpsimd.dma_start(
                consts["uiou"][:], uiou_d[:].rearrange("(kt p) m -> p kt m", p=128))
            consts["uf"] = const.tile([128, 4, 512], FR, tag="uf", name="uf")
            nc.gpsimd.dma_start(
                consts["uf"][:], uf_d[:].rearrange("(kt p) m -> p kt m", p=128))
            consts["wb"] = const.tile([128, 4], FP, tag="wb", name="wb")
            nc.sync.dma_start(
                consts["wb"][:], wb_d[:].rearrange("(m p) -> p m", p=128))
            consts["uioub"] = const.tile([128, 6], FP, tag="uioub", name="uioub")
            nc.sync.dma_start(
                consts["uioub"][:], uioub_d[:].rearrange("(m p) -> p m", p=128))
            consts["ufb"] = const.tile([128, 4], FP, tag="ufb", name="ufb")
            nc.sync.dma_start(
                consts["ufb"][:], ufb_d[:].rearrange("(m p) -> p m", p=128))

            pools = {"psum": psum, "x": xpool, "gates": gates, "out": outp}
            em = _Emitter(nc, pools, consts, out_d)

            # resident state for level CB (written by chunks, read by tail)
            wcb = 2**CB
            hcb = const.tile([128, 2, wcb], FR, tag="hcb", name="hcb")
            ccb = const.tile([128, 2, wcb], FP, tag="ccb", name="ccb")

            for ch in range(NCHUNK):
                # per-chunk state tiles, levels leaf_lvl..CB+1
                hs, cs = {}, {}
                for k in range(CB + 1, leaf_lvl + 1):
                    wk = chunk_leaves >> (leaf_lvl - k)
                    nb = 2 if k == leaf_lvl else 1
                    hs[k] = state.tile([128, 2, wk], FR, tag=f"h{k}", bufs=nb, name=f"h{k}_{ch}")
                    cs[k] = state.tile([128, 2, wk], FP, tag=f"c{k}", bufs=nb, name=f"c{k}_{ch}")
                hs[CB], cs[CB] = hcb, ccb

                em.emit_leaf_init(
                    x_d, ch * chunk_leaves, chunk_leaves,
                    hs[leaf_lvl], cs[leaf_lvl], 0,
                    (2**leaf_lvl - 1) + ch * chunk_leaves)
                for k in range(leaf_lvl - 1, CB - 1, -1):
                    wk = chunk_leaves >> (leaf_lvl - k)
                    dst0 = ch * wk if k == CB else 0
                    em.emit_level(
                        hs[k + 1], cs[k + 1], 0, hs[k], cs[k], dst0, wk,
                        (2**k - 1) + ch * wk)

            # tail: levels CB-1 .. 0
            hprev, cprev = hcb, ccb
            for k in range(CB - 1, -1, -1):
                wk = 2**k
                wa = max(wk, 4)
                hk = const.tile([128, 2, wa], FR, tag=f"ht{k}", name=f"ht{k}")
                ck = const.tile([128, 2, wa], FP, tag=f"ct{k}", name=f"ct{k}")
                if wa != wk:
                    zt = const.tile([128, 2, wa], FP, tag=f"zt{k}", name=f"zt{k}")
                    nc.vector.memset(zt[:], 0.0)
                    nc.vector.tensor_copy(hk[:], zt[:])
                    nc.vector.memset(ck[:], 0.0)
                em.emit_level(hprev, cprev, 0, hk, ck, 0, wk, 2**k - 1)
                hprev, cprev = hk, ck
            # local root c -> last output row
            em.emit_rows_out(cprev, 0, 1, lnodes)

    nc.compile()
    return nc


# ---------------- host side ----------------

_CACHE = {}
LAST_RESULTS = None


def _sigmoid(v):
    return 1.0 / (1.0 + np.exp(-v))


def _cell_np(ch_h, ch_c, Uiou_w, Uiou_b, Uf_w, Uf_b):
    f = _sigmoid(ch_h @ Uf_w + Uf_b).reshape(-1, 2, H)
    iou = ch_h @ Uiou_w + Uiou_b
    i, o, u = np.split(iou, 3, axis=1)
    i, o, u = _sigmoid(i), _sigmoid(o), np.tanh(u)
    c = i * u + (f * ch_c).sum(axis=1)
    h = o * np.tanh(c)
    return h.astype(np.float32), c.astype(np.float32)


def kernel(x, W_w, W_b, Uiou_w, Uiou_b, Uf_w, Uf_b):
    global LAST_RESULTS
    from concourse.bass_utils import run_bass_kernel_spmd

    llev = LEVELS - 3
    if "nc" not in _CACHE:
        _CACHE["nc"] = build_nc(llev)
    nc = _CACHE["nc"]

    f32 = lambda a: np.ascontiguousarray(np.asarray(a, dtype=np.float32))
    x = f32(x)
    weights = {
        "w_w": f32(W_w), "w_b": f32(W_b), "uiou_w": f32(Uiou_w),
        "uiou_b": f32(Uiou_b), "uf_w": f32(Uf_w), "ufb": f32(Uf_b),
    }
    nleaves = 2**(llev - 1)
    lnodes = 2**llev - 1
    leaf0 = 2**(LEVELS - 1) - 1
    in_maps = []
    for m in range(NCORES):
        im = dict(weights)
        im["x"] = x[leaf0 + nleaves * m: leaf0 + nleaves * (m + 1)]
        in_maps.append(im)

    res = run_bass_kernel_spmd(nc, in_maps, core_ids=list(range(NCORES)))
    LAST_RESULTS = res

    out = np.zeros((N, H), np.float32)
    roots_h = np.zeros((NCORES, H), np.float32)
    roots_c = np.zeros((NCORES, H), np.float32)
    for m in range(NCORES):
        hm = res.results[m]["out"]
        for k in range(llev):
            goff = 2**(k + 3) - 1 + m * 2**k
            loff = 2**k - 1
            out[goff:goff + 2**k] = hm[loff:loff + 2**k]
        roots_h[m] = hm[0]
        roots_c[m] = hm[lnodes]
    # top 7 nodes (global levels 2..0) on host
    h_top = np.zeros((7, H), np.float32)
    c_top = np.zeros((7, H), np.float32)
    ch_h = np.concatenate([roots_h[0::2], roots_h[1::2]], axis=1)
    ch_c = np.stack([roots_c[0::2], roots_c[1::2]], axis=1)
    h_top[3:7], c_top[3:7] = _cell_np(ch_h, ch_c, weights["uiou_w"],
                                      weights["uiou_b"], weights["uf_w"],
                                      weights["ufb"])
    for lvl in (1, 0):
        off, cnt, coff = 2**lvl - 1, 2**lvl, 2**(lvl + 1) - 1
        ch_h = h_top[coff:coff + 2 * cnt].reshape(cnt, 2 * H)
        ch_c = c_top[coff:coff + 2 * cnt].reshape(cnt, 2, H)
        h_top[off:off + cnt], c_top[off:off + cnt] = _cell_np(
            ch_h, ch_c, weights["uiou_w"], weights["uiou_b"],
            weights["uf_w"], weights["ufb"])
    out[0:7] = h_top
    return out


# revision 13
# speedup vs baseline: 2.0932x; 1.2432x over previous
"""Trainium2 Bass kernel for NTreeLSTM (complete binary tree, heap layout).

Strategy (self-contained; shapes hardcoded):
- Global tree: LEVELS=18, N=262143, H=256. Core m (of 8) owns the subtree
  rooted at global level-3 node 7+m. All child gathers are core-local.
- Only leaf rows of x matter (internal init is overwritten before use), so
  each core receives just its 16384 contiguous leaf rows of x.
- On-device state is feature-major (h^T/c^T: [feat partition, node free]) so
  biases ride the ACT engine's per-partition bias and the per-level child
  gather is a stride-2 access pattern fed straight to the PE.
- Per core: 8 chunks × (leaf init + levels 13..6), tail levels 5..0, output
  h for all 32767 local nodes (+ local root c) via PE transpose.
- Host: gathers leaf shards, scatters per-level outputs back to the global
  heap, and computes the top 7 nodes (global levels 2..0) in numpy.
"""
import sys

if "/opt/trn_rl_repo" not in sys.path:
    sys.path.insert(0, "/opt/trn_rl_repo")

import numpy as np

import concourse.bass as bass
import concourse.mybir as mybir
import concourse.tile as tile
from concourse import bacc
from concourse.masks import make_identity

FP = mybir.dt.float32
FR = mybir.dt.float32r
AF = mybir.ActivationFunctionType

H = 256
LEVELS = 18
N = 2**LEVELS - 1
NCORES = 8

G = 512          # node-group (matmul moving free dim)
NCHUNK = 8       # chunks per core
CB = 10          # resident boundary level: chunks write level CB; levels CB-1..0 run once core-wide


def _stride2(ap, parity):
    """Select every other free-dim column of a 2D [P, 2n] AP."""
    assert len(ap.ap) == 2, ap.ap
    (pstep, pcount), (fstep, fcount) = ap.ap
    assert fcount % 2 == 0
    return bass.AP(
        tensor=ap.tensor,
        offset=ap.offset + parity * fstep,
        ap=[[pstep, pcount], [2 * fstep, fcount // 2]],
    )


class _Emitter:
    def __init__(self, nc, pools, consts, out_d):
        self.nc = nc
        self.pools = pools
        self.c = consts
        self.out_d = out_d
        self._copy_flip = 0

    def _psum_copy(self, dst, src):
        # split psum->sbuf copies between DVE and ACT
        if self._copy_flip == 0:
            self.nc.vector.tensor_copy(dst, src)
        else:
            self.nc.scalar.copy(dst, src)
        self._copy_flip ^= 1

    def emit_rows_out(self, src, col0, w, row0):
        """Transpose src[:, t, col0:col0+w] (feature-major) to node-major DRAM
        rows out_d[row0:row0+w]."""
        nc, p = self.nc, self.pools
        for j0 in range(0, w, 128):
            gw = min(128, w - j0)
            ps = p["psum"].tile([128, 256], src.dtype, tag="psout", bufs=2)
            ident = self.c["identr" if src.dtype == FR else "ident"]
            nc.tensor.transpose(
                ps[:gw, 0:128], src[:, 0, col0 + j0:col0 + j0 + gw], ident)
            nc.tensor.transpose(
                ps[:gw, 128:256], src[:, 1, col0 + j0:col0 + j0 + gw], ident)
            osb = p["out"].tile([128, 256], FP, tag="osb", bufs=4)
            self._psum_copy(osb[:gw, :], ps[:gw, :])
            nc.sync.dma_start(
                self.out_d[row0 + j0:row0 + j0 + gw, :], osb[:gw, :])

    def emit_leaf_init(self, x_d, xrow0, nleaf, h14, c14, dst0, out_row0):
        """init = tanh(x @ W + b) for nleaf leaves; h/c into h14/c14 cols
        [dst0, dst0+nleaf); h also to DRAM rows [out_row0, ...)."""
        nc, p, c = self.nc, self.pools, self.c
        for g0 in range(0, nleaf, G):
            gw = min(G, nleaf - g0)
            ngr = (gw + 127) // 128
            xg = p["x"].tile([128, ngr, 256], FR, tag="xg", bufs=2)
            nc.gpsimd.dma_start(
                xg[:, :, :],
                x_d[xrow0 + g0:xrow0 + g0 + gw, :].rearrange(
                    "(j p) f -> p j f", p=128))
            xt = p["x"].tile([128, ngr, 2, 128], FR, tag="xt", bufs=2)
            for j in range(ngr):
                ps = p["psum"].tile([128, 256], FR, tag="pstr", bufs=2)
                nc.tensor.transpose(ps[:, 0:128], xg[:, j, 0:128], c["identr"])
                nc.tensor.transpose(ps[:, 128:256], xg[:, j, 128:256], c["identr"])
                self._psum_copy(xt[:, j, :, :], ps[:, :])
            for m in range(4):
                ps = p["psum"].tile([128, G], FP, tag="psmm", bufs=4)
                for kt in range(2):
                    nc.tensor.matmul(
                        ps[:, :gw],
                        c["w"][:, kt, 128 * m:128 * m + 128],
                        xt[:, :, kt, :],
                        start=(kt == 0), stop=(kt == 1))
                dst = h14 if m < 2 else c14
                t = m % 2
                nc.scalar.activation(
                    dst[:, t, dst0 + g0:dst0 + g0 + gw], ps[:, :gw],
                    AF.Tanh, bias=c["wb"][:, m:m + 1])
            self.emit_rows_out(h14, dst0 + g0, gw, out_row0 + g0)

    def emit_level(self, h_ch, c_ch, ch0, h_par, c_par, dst0, w, out_row0):
        """TreeLSTM cell for w parents whose children are cols
        [ch0, ch0+2w) of h_ch/c_ch; writes parent cols [dst0, dst0+w)."""
        nc, p, c = self.nc, self.pools, self.c
        for g0 in range(0, w, G):
            gwr = min(G, w - g0)  # true width (DMA out)
            gw = gwr + gwr % 2    # fp32r ISA needs even free counts; padded
            co = ch0 + 2 * (g0)

            def rhs(kt):
                par, t = (0, kt) if kt < 2 else (1, kt - 2)
                return _stride2(h_ch[:, t, co:co + 2 * gw], par)

            iou = p["gates"].tile([128, 6, G], FP, tag="iou", bufs=2)
            for m in range(6):
                ps = p["psum"].tile([128, G], FP, tag="psmm", bufs=4)
                for kt in range(4):
                    nc.tensor.matmul(
                        ps[:, :gw],
                        c["uiou"][:, kt, 128 * m:128 * m + 128],
                        rhs(kt), start=(kt == 0), stop=(kt == 3))
                nc.scalar.activation(
                    iou[:, m, :gw], ps[:, :gw],
                    AF.Tanh if m >= 4 else AF.Sigmoid,
                    bias=c["uioub"][:, m:m + 1])
            f = p["gates"].tile([128, 4, G], FP, tag="f", bufs=1)
            for m in range(4):
                ps = p["psum"].tile([128, G], FP, tag="psmm", bufs=4)
                for kt in range(4):
                    nc.tensor.matmul(
                        ps[:, :gw],
                        c["uf"][:, kt, 128 * m:128 * m + 128],
                        rhs(kt), start=(kt == 0), stop=(kt == 3))
                nc.scalar.activation(
                    f[:, m, :gw], ps[:, :gw], AF.Sigmoid,
                    bias=c["ufb"][:, m:m + 1])
            for t in range(2):
                cdst = c_par[:, t, dst0 + g0:dst0 + g0 + gw]
                c_ev = _stride2(c_ch[:, t, co:co + 2 * gw], 0)
                c_od = _stride2(c_ch[:, t, co:co + 2 * gw], 1)
                t1 = p["gates"].tile([128, G], FP, tag="tmp1", bufs=2)
                t2 = p["gates"].tile([128, G], FP, tag="tmp2", bufs=2)
                nc.vector.tensor_mul(cdst, f[:, t, :gw], c_ev)
                nc.vector.tensor_mul(t1[:, :gw], f[:, 2 + t, :gw], c_od)
                nc.vector.tensor_mul(t2[:, :gw], iou[:, t, :gw], iou[:, 4 + t, :gw])
                nc.vector.tensor_add(cdst, cdst, t1[:, :gw])
                nc.vector.tensor_add(cdst, cdst, t2[:, :gw])
                th = p["gates"].tile([128, G], FP, tag="tanhc", bufs=2)
                nc.scalar.activation(th[:, :gw], cdst, AF.Tanh)
                nc.vector.tensor_mul(
                    h_par[:, t, dst0 + g0:dst0 + g0 + gw],
                    iou[:, 2 + t, :gw], th[:, :gw])
            self.emit_rows_out(h_par, dst0 + g0, gwr, out_row0 + g0)


def build_nc(llev=15):
    """Build the single-core program. Local tree: levels 0..llev-1, leaves at
    local level llev-1; identical on all 8 cores (SPMD, no collectives)."""
    leaf_lvl = llev - 1
    nleaves = 2**leaf_lvl
    lnodes = 2**llev - 1
    chunk_leaves = nleaves // NCHUNK

    nc = bacc.Bacc(None, target_bir_lowering=False)
    x_d = nc.dram_tensor("x", [nleaves, 256], FP, kind="ExternalInput")
    w_d = nc.dram_tensor("w_w", [256, 512], FP, kind="ExternalInput")
    wb_d = nc.dram_tensor("w_b", [512], FP, kind="ExternalInput")
    uiou_d = nc.dram_tensor("uiou_w", [512, 768], FP, kind="ExternalInput")
    uioub_d = nc.dram_tensor("uiou_b", [768], FP, kind="ExternalInput")
    uf_d = nc.dram_tensor("uf_w", [512, 512], FP, kind="ExternalInput")
    ufb_d = nc.dram_tensor("ufb", [512], FP, kind="ExternalInput")
    out_d = nc.dram_tensor("out", [lnodes + 1, 256], FP, kind="ExternalOutput")

    with tile.TileContext(nc) as tc:
        with (
            tc.tile_pool(name="const", bufs=1) as const,
            tc.tile_pool(name="state", bufs=1) as state,
            tc.tile_pool(name="x", bufs=1) as xpool,
            tc.tile_pool(name="gates", bufs=1) as gates,
            tc.tile_pool(name="out", bufs=1) as outp,
            tc.tile_pool(name="psum", bufs=1, space="PSUM") as psum,
        ):
            consts = {}
            consts["ident"] = const.tile([128, 128], FP, tag="ident", name="ident")
            make_identity(nc, consts["ident"])
            consts["identr"] = const.tile([128, 128], FR, tag="identr", name="identr")
            nc.vector.tensor_copy(consts["identr"][:], consts["ident"][:])
            consts["w"] = const.tile([128, 2, 512], FR, tag="w", name="w")
            nc.gpsimd.dma_start(
                consts["w"][:], w_d[:].rearrange("(kt p) m -> p kt m", p=128))
            consts["uiou"] = const.tile([128, 4, 768], FR, tag="uiou", name="uiou")
            nc.gpsimd.dma_start(
                consts["uiou"][:], uiou_d[:].rearrange("(kt p) m -> p kt m", p=128))
            consts["uf"] = const.tile([128, 4, 512], FR, tag="uf", name="uf")
            nc.gpsimd.dma_start(
                consts["uf"][:], uf_d[:].rearrange("(kt p) m -> p kt m", p=128))
            consts["wb"] = const.tile([128, 4], FP, tag="wb", name="wb")
            nc.sync.dma_start(
                consts["wb"][:], wb_d[:].rearrange("(m p) -> p m", p=128))
            consts["uioub"] = const.tile([128, 6], FP, tag="uioub", name="uioub")
            nc.sync.dma_start(
                consts["uioub"][:], uioub_d[:].rearrange("(m p) -> p m", p=128))
            consts["ufb"] = const.tile([128, 4], FP, tag="ufb", name="ufb")
            nc.sync.dma_start(
                consts["ufb"][:], ufb_d[:].rearrange("(m p) -> p m", p=128))

            pools = {"psum": psum, "x": xpool, "gates": gates, "out": outp}
            em = _Emitter(nc, pools, consts, out_d)

            # resident whole-core state for level RB (written per chunk)
            RB = CB
            wrb = 2**RB
            hcb = const.tile([128, 2, wrb], FR, tag="hcb", name="hcb")
            ccb = const.tile([128, 2, wrb], FP, tag="ccb", name="ccb")

            chunk_rb = wrb // NCHUNK
            for ch in range(NCHUNK):
                # per-chunk state tiles, levels leaf_lvl..RB+1
                hs, cs = {}, {}
                for k in range(RB + 1, leaf_lvl + 1):
                    wk = chunk_leaves >> (leaf_lvl - k)
                    nb = 1
                    hs[k] = state.tile([128, 2, wk], FR, tag=f"h{k}", bufs=nb, name=f"h{k}_{ch}")
                    cs[k] = state.tile([128, 2, wk], FP, tag=f"c{k}", bufs=nb, name=f"c{k}_{ch}")
                hs[RB], cs[RB] = hcb, ccb

                em.emit_leaf_init(
                    x_d, ch * chunk_leaves, chunk_leaves,
                    hs[leaf_lvl], cs[leaf_lvl], 0,
                    (2**leaf_lvl - 1) + ch * chunk_leaves)
                for k in range(leaf_lvl - 1, RB - 1, -1):
                    wk = chunk_leaves >> (leaf_lvl - k)
                    dst0 = ch * wk if k == RB else 0
                    em.emit_level(
                        hs[k + 1], cs[k + 1], 0, hs[k], cs[k], dst0, wk,
                        (2**k - 1) + ch * wk)

            # resident low levels: RB-1 .. 0, processed once core-wide
            hprev, cprev = hcb, ccb
            for k in range(RB - 1, -1, -1):
                wk = 2**k
                wa = max(wk, 4)
                hk = const.tile([128, 2, wa], FR, tag=f"ht{k}", name=f"ht{k}")
                ck = const.tile([128, 2, wa], FP, tag=f"ct{k}", name=f"ct{k}")
                if wa != wk:
                    zt = const.tile([128, 2, wa], FP, tag=f"zt{k}", name=f"zt{k}")
                    nc.vector.memset(zt[:], 0.0)
                    nc.vector.tensor_copy(hk[:], zt[:])
                    nc.vector.memset(ck[:], 0.0)
                em.emit_level(hprev, cprev, 0, hk, ck, 0, wk, 2**k - 1)
                hprev, cprev = hk, ck
            # local root c -> last output row
            em.emit_rows_out(cprev, 0, 1, lnodes)

    nc.compile()
    return nc


# ---------------- host side ----------------

_CACHE = {}
LAST_RESULTS = None


def _sigmoid(v):
    return 1.0 / (1.0 + np.exp(-v))


def _cell_np(ch_h, ch_c, Uiou_w, Uiou_b, Uf_w, Uf_b):
    f = _sigmoid(ch_h @ Uf_w + Uf_b).reshape(-1, 2, H)
    iou = ch_h @ Uiou_w + Uiou_b
    i, o, u = np.split(iou, 3, axis=1)
    i, o, u = _sigmoid(i), _sigmoid(o), np.tanh(u)
    c = i * u + (f * ch_c).sum(axis=1)
    h = o * np.tanh(c)
    return h.astype(np.float32), c.astype(np.float32)


def kernel(x, W_w, W_b, Uiou_w, Uiou_b, Uf_w, Uf_b):
    global LAST_RESULTS
    from concourse.bass_utils import run_bass_kernel_spmd

    llev = LEVELS - 3
    if "nc" not in _CACHE:
        _CACHE["nc"] = build_nc(llev)
    nc = _CACHE["nc"]

    f32 = lambda a: np.ascontiguousarray(np.asarray(a, dtype=np.float32))
    x = f32(x)
    weights = {
        "w_w": f32(W_w), "w_b": f32(W_b), "uiou_w": f32(Uiou_w),
        "uiou_b": f32(Uiou_b), "uf_w": f32(Uf_w), "ufb": f32(Uf_b),
    }
    nleaves = 2**(llev - 1)
    lnodes = 2**llev - 1
    leaf0 = 2**(LEVELS - 1) - 1
    in_maps = []
    for m in range(NCORES):
        im = dict(weights)
        im["x"] = x[leaf0 + nleaves * m: leaf0 + nleaves * (m + 1)]
        in_maps.append(im)

    res = run_bass_kernel_spmd(nc, in_maps, core_ids=list(range(NCORES)))
    LAST_RESULTS = res

    out = np.zeros((N, H), np.float32)
    roots_h = np.zeros((NCORES, H), np.float32)
    roots_c = np.zeros((NCORES, H), np.float32)
    for m in range(NCORES):
        hm = res.results[m]["out"]
        for k in range(llev):
            goff = 2**(k + 3) - 1 + m * 2**k
            loff = 2**k - 1
            out[goff:goff + 2**k] = hm[loff:loff + 2**k]
        roots_h[m] = hm[0]
        roots_c[m] = hm[lnodes]
    # top 7 nodes (global levels 2..0) on host
    h_top = np.zeros((7, H), np.float32)
    c_top = np.zeros((7, H), np.float32)
    ch_h = np.concatenate([roots_h[0::2], roots_h[1::2]], axis=1)
    ch_c = np.stack([roots_c[0::2], roots_c[1::2]], axis=1)
    h_top[3:7], c_top[3:7] = _cell_np(ch_h, ch_c, weights["uiou_w"],
                                      weights["uiou_b"], weights["uf_w"],
                                      weights["ufb"])
    for lvl in (1, 0):
        off, cnt, coff = 2**lvl - 1, 2**lvl, 2**(lvl + 1) - 1
        ch_h = h_top[coff:coff + 2 * cnt].reshape(cnt, 2 * H)
        ch_c = c_top[coff:coff + 2 * cnt].reshape(cnt, 2, H)
        h_top[off:off + cnt], c_top[off:off + cnt] = _cell_np(
            ch_h, ch_c, weights["uiou_w"], weights["uiou_b"],
            weights["uf_w"], weights["ufb"])
    out[0:7] = h_top
    return out
